# revision 40
# baseline (speedup 1.0000x reference)
"""BilateralGridLayer Trainium2 kernel.

Data-parallel over batch: sample b -> NeuronCore b (8 cores).
Per core, for its (3, 512, 512) sample:
  - wz_full: per-pixel Gaussian soft-binning of channel 2 over a 16-bin grid.
    One Derivative_Erf activation per bin (erf'(x) = 2/sqrt(pi)*exp(-x^2);
    the constant cancels in the normalization), then DVE reduce+recip+mul.
  - wx_full / wy_full: broadcasts of the tiny spatial soft-bin table,
    materialized from SBUF-resident broadcast tiles at DMA line rate.
  - grid_value: weighted reductions using ones-stationary matmuls on the
    TensorEngine; the z-term reduces a bf16 product tile on the PE.
Modeled (instruction-cost timeline) per-core time ~178 us, ~90% of the
~161 us DMA floor for the 55 MB/core of HBM traffic this layer moves.
"""

import numpy as np

GRID_SIZE = 16
SIGMA_SPATIAL = 0.1
SIGMA_COLOR = 0.1
B, C, H, W = 8, 3, 512, 512
G = GRID_SIZE
P = 128          # SBUF partitions
NH = H // P      # 4 row blocks (h-chunks)
S = 256          # pixels per partition per sub-chunk (w-half)
NW = W // S      # 2 w-halves
N_CORES = 8

_NC = None            # cached Bass module
_LAST_RESULTS = None  # BassKernelResults of the most recent run (for test.py)

# tile-pool buffer counts (module-level so perf sweeps can override)
BUFS = {"xio": 2, "ep": 2, "tmpbp": 2, "wybp": 2, "small": 2}


def _soft_bin_table(n: int, sigma: float) -> np.ndarray:
    """Host-side mirror of reference._soft_bin for the spatial coords."""
    c = np.linspace(0.0, 1.0, n, dtype=np.float32)
    g = np.linspace(0.0, 1.0, G, dtype=np.float32)
    d = c[:, None] - g[None, :]
    w = np.exp(-(d * d) / np.float32(2.0 * sigma * sigma))
    return (w / w.sum(axis=1, keepdims=True)).astype(np.float32)


def _build_nc(repeat: int = 1):
    import concourse.bacc as bacc
    import concourse.mybir as mybir
    from concourse.tile import TileContext

    f32 = mybir.dt.float32
    bf16 = mybir.dt.bfloat16
    AF = mybir.ActivationFunctionType
    AX = mybir.AxisListType

    nc = bacc.Bacc("TRN2", target_bir_lowering=False, debug=False)
    xb = nc.declare_dram_parameter("xb", [C, H, W], f32, isOutput=False)
    wtab = nc.declare_dram_parameter("wtab", [H, G], f32, isOutput=False)
    wrow = nc.declare_dram_parameter("wrow", [P, W * G], f32, isOutput=False)
    gv = nc.declare_dram_parameter("gv", [1, G], f32, isOutput=True)
    owx = nc.declare_dram_parameter("owx", [H, W, G], f32, isOutput=True)
    owy = nc.declare_dram_parameter("owy", [H, W, G], f32, isOutput=True)
    owz = nc.declare_dram_parameter("owz", [H, W, G], f32, isOutput=True)

    gvals = np.linspace(0.0, 1.0, G, dtype=np.float32)
    # exp(-(v-g)^2/(2s^2)) = (sqrt(pi)/2) * erf'(k*(v-g)), k = 1/(s*sqrt(2)).
    # The sqrt(pi)/2 factor cancels in the normalization, so one
    # Derivative_Erf activation per bin computes the (unscaled) weights.
    erf_scale = float(np.float32(1.0 / (SIGMA_COLOR * np.sqrt(2.0))))

    with TileContext(nc) as tc:
        with (
            tc.tile_pool(name="const", bufs=1) as constp,
            tc.tile_pool(name="xio", bufs=BUFS["xio"]) as xio,
            tc.tile_pool(name="tmpbp", bufs=BUFS["tmpbp"]) as tmpbp,
            tc.tile_pool(name="ep", bufs=BUFS["ep"]) as ep,
            tc.tile_pool(name="wybp", bufs=BUFS["wybp"]) as wybp,
            tc.tile_pool(name="small", bufs=BUFS["small"]) as smallp,
            tc.tile_pool(name="psum", bufs=1, space="PSUM") as psump,
            tc.tile_pool(name="dram", bufs=1, space="DRAM") as dramp,
        ):
          for _rep in range(repeat):
            # ---- constants, loaded once ----
            wrow_sb = constp.tile([P, W * G], f32)
            nc.sync.dma_start(out=wrow_sb[:], in_=wrow[:])
            wtab4 = constp.tile([P, NH, G], f32)
            nc.sync.dma_start(
                out=wtab4[:], in_=wtab[:].rearrange("(c p) g -> p c g", p=P)
            )
            ones = constp.tile([P, 1], f32)
            nc.vector.memset(ones[:], 1.0)
            onesb = constp.tile([P, 1], bf16)
            nc.vector.memset(onesb[:], 1.0)
            dbias = constp.tile([P, G], f32)
            for gi in range(G):
                nc.vector.memset(
                    dbias[:, gi : gi + 1],
                    -float(np.float32(erf_scale) * np.float32(gvals[gi])),
                )

            # prefetch whole channels: tile[p, c*W+w] = x[ch, c*P+p, w]
            xch = []
            for ch in range(C):
                xc_t = xio.tile([P, NH * W], f32, name=f"xch{ch}", tag=f"xch{ch}")
                nc.sync.dma_start(
                    out=xc_t[:].rearrange("p (c w) -> p c w", w=W),
                    in_=xb[ch, :, :].rearrange("(c p) w -> p c w", p=P),
                )
                xch.append(xc_t)

            # ---- PSUM accumulators ----
            s_ps = psump.tile([1, W], f32)      # column sums of x0
            typ = psump.tile([1, G], f32)       # term_y
            txp = psump.tile([1, G], f32)       # term_x
            tzp = psump.tile([1, W], f32)       # term_z partials (32 fblk x 16 g)

            tz_started = [False]

            for c in range(NH):
                rows = slice(c * P, (c + 1) * P)

                x0c = xch[0][:, c * W : (c + 1) * W]
                x1c = xch[1][:, c * W : (c + 1) * W]
                x2c = xch[2][:, c * W : (c + 1) * W]

                # term_x partial: S[w] += sum_h x0[h, w]  (ones-stationary)
                nc.tensor.matmul(
                    s_ps[:], lhsT=ones[:], rhs=x0c,
                    start=(c == 0), stop=(c == NH - 1),
                )
                # term_y partial: row sums of x1, dotted with the h table
                s2c = smallp.tile([P, 1], f32, tag="s2")
                nc.vector.reduce_sum(out=s2c[:], in_=x1c, axis=AX.X)
                nc.tensor.matmul(
                    typ[:], lhsT=s2c[:], rhs=wtab4[:, c, :],
                    start=(c == 0), stop=(c == NH - 1),
                )

                if c == NH - 1:
                    # term_x tail: S (1,512) -> DRAM -> (128,4) partition-major.
                    # Emitted here (right after the last colsum matmul) so the
                    # roundtrip overlaps the final chunk's main compute.
                    s_sb = smallp.tile([1, W], f32, tag="ssb")
                    nc.scalar.copy(out=s_sb[:], in_=s_ps[:])
                    stag = dramp.tile([1, W], f32)
                    nc.sync.dma_start(out=stag[:], in_=s_sb[:])
                    sT = smallp.tile([P, NH], f32, tag="sT")
                    nc.sync.dma_start(
                        out=sT[:], in_=stag[0, :].rearrange("(cc p) -> p cc", p=P)
                    )
                    for jc in range(NH):
                        nc.tensor.matmul(
                            txp[:], lhsT=sT[:, jc : jc + 1], rhs=wtab4[:, jc, :],
                            start=(jc == 0), stop=(jc == NH - 1),
                        )

                # wy_full row block: per-partition h-row table broadcast along w
                wyb = wybp.tile([P, S * G], f32, tag="wyb")
                nc.gpsimd.tensor_copy(
                    out=wyb[:].rearrange("p (s g) -> p s g", g=G),
                    in_=wtab4[:, c : c + 1, :].broadcast_to((P, S, G)),
                )
                for j in range(NW):
                    nc.sync.dma_start(
                        out=owy[rows, j * S : (j + 1) * S, :].rearrange(
                            "p w g -> p (w g)"
                        ),
                        in_=wyb[:],
                    )
                # wx_full row block straight from the resident broadcast tile
                nc.sync.dma_start(
                    out=owx[rows, :, :].rearrange("p w g -> p (w g)"),
                    in_=wrow_sb[:],
                )

                segs = [S] * NW
                col0 = 0
                for j, Sj in enumerate(segs):
                    cols = slice(col0, col0 + Sj)
                    col0 += Sj
                    x2h = x2c[:, cols]
                    e = ep.tile([P, S * G], f32, tag="e")
                    e3 = e[:, : Sj * G].rearrange("p (s g) -> p s g", g=G)
                    for gi in range(G):
                        nc.scalar.activation(
                            e3[:, :, gi], x2h, AF.Derivative_Erf,
                            bias=dbias[:, gi : gi + 1], scale=erf_scale,
                        )
                    s_t = smallp.tile([P, S], f32, tag="s")
                    nc.vector.reduce_sum(out=s_t[:, :Sj], in_=e3, axis=AX.X)
                    rcp = smallp.tile([P, S], f32, tag="rcp")
                    nc.vector.reciprocal(rcp[:, :Sj], s_t[:, :Sj])
                    q = smallp.tile([P, S], f32, tag="q")
                    nc.vector.tensor_mul(q[:, :Sj], rcp[:, :Sj], x2h)

                    # term_z: tmpb = e * q (bf16), then PE-reduce over partitions
                    tmpb = tmpbp.tile([P, S * G], bf16, tag="tmpb")
                    nc.vector.tensor_mul(
                        tmpb[:, : Sj * G].rearrange("p (s g) -> p s g", g=G),
                        e3,
                        q[:, :Sj, None].broadcast_to((P, Sj, G)),
                    )
                    nblk = (Sj * G) // W  # 512-wide matmul blocks
                    for k in range(nblk):
                        last = (
                            (c == NH - 1)
                            and (j == len(segs) - 1)
                            and (k == nblk - 1)
                        )
                        nc.tensor.matmul(
                            tzp[:], lhsT=onesb[:],
                            rhs=tmpb[:, k * W : (k + 1) * W],
                            start=(not tz_started[0]), stop=last,
                        )
                        tz_started[0] = True

                    # normalize in place and store
                    nc.vector.tensor_mul(
                        e3, e3, rcp[:, :Sj, None].broadcast_to((P, Sj, G))
                    )
                    nc.sync.dma_start(out=owz[rows, cols, :], in_=e3)

            # ---- final grid_value assembly ----
            # term_z: fold the 32 f-blocks of tzp into 16 g values
            tzs = smallp.tile([1, W], f32, tag="tzs")
            nc.scalar.copy(out=tzs[:], in_=tzp[:])
            tzv = smallp.tile([1, G], f32, tag="tzv")
            tzs3 = tzs[:].rearrange("a (r g) -> a g r", g=G)
            for gi in range(G):
                nc.vector.reduce_sum(
                    out=tzv[:, gi : gi + 1], in_=tzs3[:, gi, :], axis=AX.X
                )

            gv_sb = smallp.tile([1, G], f32, tag="gv")
            nc.scalar.copy(out=gv_sb[:], in_=txp[:])
            nc.vector.tensor_add(gv_sb[:], gv_sb[:], typ[:])
            nc.vector.tensor_add(gv_sb[:], gv_sb[:], tzv[:])
            nc.sync.dma_start(out=gv[:], in_=gv_sb[:])

    nc.compile()
    return nc


def _get_nc():
    global _NC
    if _NC is None:
        _NC = _build_nc()
    return _NC


def kernel(x: np.ndarray):
    global _LAST_RESULTS
    from concourse.bass_utils import run_bass_kernel_spmd

    x = np.ascontiguousarray(np.asarray(x), dtype=np.float32)
    assert x.shape == (B, C, H, W), x.shape

    nc = _get_nc()
    wtab = _soft_bin_table(H, SIGMA_SPATIAL)                      # (512, 16)
    wrow = np.ascontiguousarray(np.tile(wtab.reshape(1, H * G), (P, 1)))
    in_maps = [{"xb": x[b], "wtab": wtab, "wrow": wrow} for b in range(B)]

    res = run_bass_kernel_spmd(nc, in_maps, list(range(N_CORES)))
    _LAST_RESULTS = res
    outs = res.results

    grid_value = np.stack([o["gv"] for o in outs]).astype(np.float32)  # (8,1,16)
    wx_full = np.stack([o["owx"] for o in outs])[:, None]
    wy_full = np.stack([o["owy"] for o in outs])[:, None]
    wz_full = np.stack([o["owz"] for o in outs])[:, None]
    return grid_value, wx_full, wy_full, wz_full


# revision 47
# speedup vs baseline: 1.0045x; 1.0045x over previous
"""BilateralGridLayer Trainium2 kernel.

Data-parallel over batch: sample b -> NeuronCore b (8 cores).
Per core, for its (3, 512, 512) sample:
  - wz_full: per-pixel Gaussian soft-binning of channel 2 over a 16-bin grid.
    One Derivative_Erf activation per bin (erf'(x) = 2/sqrt(pi)*exp(-x^2);
    the constant cancels in the normalization), then DVE reduce+recip+mul.
  - wx_full / wy_full: broadcasts of the tiny spatial soft-bin table,
    materialized from SBUF-resident broadcast tiles at DMA line rate.
  - grid_value: weighted reductions using ones-stationary matmuls on the
    TensorEngine; the z-term reduces a bf16 product tile on the PE.
Modeled (instruction-cost timeline) per-core time ~178 us, ~90% of the
~161 us DMA floor for the 55 MB/core of HBM traffic this layer moves.
"""

import numpy as np

GRID_SIZE = 16
SIGMA_SPATIAL = 0.1
SIGMA_COLOR = 0.1
B, C, H, W = 8, 3, 512, 512
G = GRID_SIZE
P = 128          # SBUF partitions
NH = H // P      # 4 row blocks (h-chunks)
S = 256          # pixels per partition per sub-chunk (w-half)
NW = W // S      # 2 w-halves
N_CORES = 8

_NC = None            # cached Bass module
_LAST_RESULTS = None  # BassKernelResults of the most recent run (for test.py)

# tile-pool buffer counts (module-level so perf sweeps can override)
BUFS = {"xio": 2, "ep": 2, "tmpbp": 2, "wybp": 2, "small": 2}


def _soft_bin_table(n: int, sigma: float) -> np.ndarray:
    """Host-side mirror of reference._soft_bin for the spatial coords."""
    c = np.linspace(0.0, 1.0, n, dtype=np.float32)
    g = np.linspace(0.0, 1.0, G, dtype=np.float32)
    d = c[:, None] - g[None, :]
    w = np.exp(-(d * d) / np.float32(2.0 * sigma * sigma))
    return (w / w.sum(axis=1, keepdims=True)).astype(np.float32)


def _build_nc(repeat: int = 1):
    import concourse.bacc as bacc
    import concourse.mybir as mybir
    from concourse.tile import TileContext

    f32 = mybir.dt.float32
    bf16 = mybir.dt.bfloat16
    AF = mybir.ActivationFunctionType
    AX = mybir.AxisListType

    nc = bacc.Bacc("TRN2", target_bir_lowering=False, debug=False)
    xb = nc.declare_dram_parameter("xb", [C, H, W], f32, isOutput=False)
    wtab = nc.declare_dram_parameter("wtab", [H, G], f32, isOutput=False)
    gv = nc.declare_dram_parameter("gv", [1, G], f32, isOutput=True)
    owx = nc.declare_dram_parameter("owx", [H, W, G], f32, isOutput=True)
    owy = nc.declare_dram_parameter("owy", [H, W, G], f32, isOutput=True)
    owz = nc.declare_dram_parameter("owz", [H, W, G], f32, isOutput=True)

    gvals = np.linspace(0.0, 1.0, G, dtype=np.float32)
    # exp(-(v-g)^2/(2s^2)) = (sqrt(pi)/2) * erf'(k*(v-g)), k = 1/(s*sqrt(2)).
    # The sqrt(pi)/2 factor cancels in the normalization, so one
    # Derivative_Erf activation per bin computes the (unscaled) weights.
    erf_scale = float(np.float32(1.0 / (SIGMA_COLOR * np.sqrt(2.0))))

    with TileContext(nc) as tc:
        with (
            tc.tile_pool(name="const", bufs=1) as constp,
            tc.tile_pool(name="xio", bufs=BUFS["xio"]) as xio,
            tc.tile_pool(name="tmpbp", bufs=BUFS["tmpbp"]) as tmpbp,
            tc.tile_pool(name="ep", bufs=BUFS["ep"]) as ep,
            tc.tile_pool(name="wybp", bufs=BUFS["wybp"]) as wybp,
            tc.tile_pool(name="small", bufs=BUFS["small"]) as smallp,
            tc.tile_pool(name="psum", bufs=1, space="PSUM") as psump,
            tc.tile_pool(name="dram", bufs=1, space="DRAM") as dramp,
        ):
          for _rep in range(repeat):
            # ---- constants, loaded once ----
            wtab4 = constp.tile([P, NH, G], f32)
            nc.sync.dma_start(
                out=wtab4[:], in_=wtab[:].rearrange("(c p) g -> p c g", p=P)
            )
            ones = constp.tile([P, 1], f32)
            nc.vector.memset(ones[:], 1.0)
            onesb = constp.tile([P, 1], bf16)
            nc.vector.memset(onesb[:], 1.0)

            # wx broadcast tile built on-chip: load the flat 32KB table onto
            # one partition, then PE-broadcast to all 128 partitions via a
            # ones-stationary K=1 matmul (single-term sums -> bit-exact copy).
            wrow_sb = constp.tile([P, W * G], f32)
            wrow1 = constp.tile([1, W * G], f32)
            nc.sync.dma_start(out=wrow1[:], in_=wtab[:].rearrange("h g -> (h g)"))
            ones1r = constp.tile([1, P], f32)
            nc.vector.memset(ones1r[:], 1.0)
            NB = 512  # one PSUM bank per broadcast block
            bcast_ps = [
                psump.tile([P, NB], f32, name=f"bc{_rep}_{i}") for i in range(2)
            ]
            for k in range((W * G) // NB):
                bp = bcast_ps[k % 2]
                nc.tensor.matmul(
                    bp[:], lhsT=ones1r[:],
                    rhs=wrow1[:, k * NB : (k + 1) * NB],
                    start=True, stop=True,
                )
                nc.scalar.copy(
                    out=wrow_sb[:, k * NB : (k + 1) * NB], in_=bp[:]
                )
            dbias = constp.tile([P, G], f32)
            for gi in range(G):
                nc.vector.memset(
                    dbias[:, gi : gi + 1],
                    -float(np.float32(erf_scale) * np.float32(gvals[gi])),
                )

            # prefetch whole channels: tile[p, c*W+w] = x[ch, c*P+p, w]
            xch = []
            for ch in range(C):
                xc_t = xio.tile([P, NH * W], f32, name=f"xch{ch}", tag=f"xch{ch}")
                nc.sync.dma_start(
                    out=xc_t[:].rearrange("p (c w) -> p c w", w=W),
                    in_=xb[ch, :, :].rearrange("(c p) w -> p c w", p=P),
                )
                xch.append(xc_t)

            # ---- PSUM accumulators ----
            s_ps = psump.tile([1, W], f32)      # column sums of x0
            typ = psump.tile([1, G], f32)       # term_y
            txp = psump.tile([1, G], f32)       # term_x
            tzp = psump.tile([1, W], f32)       # term_z partials (32 fblk x 16 g)

            tz_started = [False]

            for c in range(NH):
                rows = slice(c * P, (c + 1) * P)

                x0c = xch[0][:, c * W : (c + 1) * W]
                x1c = xch[1][:, c * W : (c + 1) * W]
                x2c = xch[2][:, c * W : (c + 1) * W]

                # term_x partial: S[w] += sum_h x0[h, w]  (ones-stationary)
                nc.tensor.matmul(
                    s_ps[:], lhsT=ones[:], rhs=x0c,
                    start=(c == 0), stop=(c == NH - 1),
                )
                # term_y partial: row sums of x1, dotted with the h table
                s2c = smallp.tile([P, 1], f32, tag="s2")
                nc.vector.reduce_sum(out=s2c[:], in_=x1c, axis=AX.X)
                nc.tensor.matmul(
                    typ[:], lhsT=s2c[:], rhs=wtab4[:, c, :],
                    start=(c == 0), stop=(c == NH - 1),
                )

                if c == NH - 1:
                    # term_x tail: S (1,512) -> DRAM -> (128,4) partition-major.
                    # Emitted here (right after the last colsum matmul) so the
                    # roundtrip overlaps the final chunk's main compute.
                    s_sb = smallp.tile([1, W], f32, tag="ssb")
                    nc.scalar.copy(out=s_sb[:], in_=s_ps[:])
                    stag = dramp.tile([1, W], f32)
                    nc.sync.dma_start(out=stag[:], in_=s_sb[:])
                    sT = smallp.tile([P, NH], f32, tag="sT")
                    nc.sync.dma_start(
                        out=sT[:], in_=stag[0, :].rearrange("(cc p) -> p cc", p=P)
                    )
                    for jc in range(NH):
                        nc.tensor.matmul(
                            txp[:], lhsT=sT[:, jc : jc + 1], rhs=wtab4[:, jc, :],
                            start=(jc == 0), stop=(jc == NH - 1),
                        )

                # wy_full row block: per-partition h-row table broadcast along w
                wyb = wybp.tile([P, S * G], f32, tag="wyb")
                nc.gpsimd.tensor_copy(
                    out=wyb[:].rearrange("p (s g) -> p s g", g=G),
                    in_=wtab4[:, c : c + 1, :].broadcast_to((P, S, G)),
                )
                for j in range(NW):
                    nc.sync.dma_start(
                        out=owy[rows, j * S : (j + 1) * S, :].rearrange(
                            "p w g -> p (w g)"
                        ),
                        in_=wyb[:],
                    )
                # wx_full row block straight from the resident broadcast tile
                nc.sync.dma_start(
                    out=owx[rows, :, :].rearrange("p w g -> p (w g)"),
                    in_=wrow_sb[:],
                )

                segs = [S] * NW
                col0 = 0
                for j, Sj in enumerate(segs):
                    cols = slice(col0, col0 + Sj)
                    col0 += Sj
                    x2h = x2c[:, cols]
                    e = ep.tile([P, S * G], f32, tag="e")
                    e3 = e[:, : Sj * G].rearrange("p (s g) -> p s g", g=G)
                    for gi in range(G):
                        nc.scalar.activation(
                            e3[:, :, gi], x2h, AF.Derivative_Erf,
                            bias=dbias[:, gi : gi + 1], scale=erf_scale,
                        )
                    s_t = smallp.tile([P, S], f32, tag="s")
                    nc.vector.reduce_sum(out=s_t[:, :Sj], in_=e3, axis=AX.X)
                    rcp = smallp.tile([P, S], f32, tag="rcp")
                    nc.vector.reciprocal(rcp[:, :Sj], s_t[:, :Sj])
                    q = smallp.tile([P, S], f32, tag="q")
                    nc.vector.tensor_mul(q[:, :Sj], rcp[:, :Sj], x2h)

                    # term_z: tmpb = e * q (bf16), then PE-reduce over partitions
                    tmpb = tmpbp.tile([P, S * G], bf16, tag="tmpb")
                    nc.vector.tensor_mul(
                        tmpb[:, : Sj * G].rearrange("p (s g) -> p s g", g=G),
                        e3,
                        q[:, :Sj, None].broadcast_to((P, Sj, G)),
                    )
                    nblk = (Sj * G) // W  # 512-wide matmul blocks
                    for k in range(nblk):
                        last = (
                            (c == NH - 1)
                            and (j == len(segs) - 1)
                            and (k == nblk - 1)
                        )
                        nc.tensor.matmul(
                            tzp[:], lhsT=onesb[:],
                            rhs=tmpb[:, k * W : (k + 1) * W],
                            start=(not tz_started[0]), stop=last,
                        )
                        tz_started[0] = True

                    # normalize in place and store
                    nc.vector.tensor_mul(
                        e3, e3, rcp[:, :Sj, None].broadcast_to((P, Sj, G))
                    )
                    nc.sync.dma_start(out=owz[rows, cols, :], in_=e3)

            # ---- final grid_value assembly ----
            # term_z: fold the 32 f-blocks of tzp into 16 g values
            tzs = smallp.tile([1, W], f32, tag="tzs")
            nc.scalar.copy(out=tzs[:], in_=tzp[:])
            tzv = smallp.tile([1, G], f32, tag="tzv")
            tzs3 = tzs[:].rearrange("a (r g) -> a g r", g=G)
            for gi in range(G):
                nc.vector.reduce_sum(
                    out=tzv[:, gi : gi + 1], in_=tzs3[:, gi, :], axis=AX.X
                )

            gv_sb = smallp.tile([1, G], f32, tag="gv")
            nc.scalar.copy(out=gv_sb[:], in_=txp[:])
            nc.vector.tensor_add(gv_sb[:], gv_sb[:], typ[:])
            nc.vector.tensor_add(gv_sb[:], gv_sb[:], tzv[:])
            nc.sync.dma_start(out=gv[:], in_=gv_sb[:])

    nc.compile()
    return nc


def _get_nc():
    global _NC
    if _NC is None:
        _NC = _build_nc()
    return _NC


def kernel(x: np.ndarray):
    global _LAST_RESULTS
    from concourse.bass_utils import run_bass_kernel_spmd

    x = np.ascontiguousarray(np.asarray(x), dtype=np.float32)
    assert x.shape == (B, C, H, W), x.shape

    nc = _get_nc()
    wtab = _soft_bin_table(H, SIGMA_SPATIAL)                      # (512, 16)
    in_maps = [{"xb": x[b], "wtab": wtab} for b in range(B)]

    res = run_bass_kernel_spmd(nc, in_maps, list(range(N_CORES)))
    _LAST_RESULTS = res
    outs = res.results

    grid_value = np.stack([o["gv"] for o in outs]).astype(np.float32)  # (8,1,16)
    wx_full = np.stack([o["owx"] for o in outs])[:, None]
    wy_full = np.stack([o["owy"] for o in outs])[:, None]
    wz_full = np.stack([o["owz"] for o in outs])[:, None]
    return grid_value, wx_full, wy_full, wz_full


# revision 52
# speedup vs baseline: 1.0962x; 1.0913x over previous
"""BilateralGridLayer Trainium2 kernel.

Data-parallel over batch: sample b -> NeuronCore b (8 cores).
Per core, for its (3, 512, 512) sample:
  - wz_full: per-pixel Gaussian soft-binning of channel 2 over a 16-bin grid.
    One Derivative_Erf activation per bin (erf'(x) = 2/sqrt(pi)*exp(-x^2);
    the constant cancels in the normalization), then DVE reduce+recip+mul.
  - wx_full / wy_full: broadcasts of the tiny spatial soft-bin table,
    materialized from SBUF-resident broadcast tiles at DMA line rate.
  - grid_value: weighted reductions using ones-stationary matmuls on the
    TensorEngine; the z-term reduces a bf16 product tile on the PE.
Modeled (instruction-cost timeline) per-core time ~178 us against a
~149 us DMA floor for the 51 MB/core of HBM traffic this layer moves
(48 MB of that is the mandatory output writes; the replicated wx table
is broadcast on-chip via K=1 ones-matmuls instead of a 4 MB DMA load).
"""

import numpy as np

GRID_SIZE = 16
SIGMA_SPATIAL = 0.1
SIGMA_COLOR = 0.1
B, C, H, W = 8, 3, 512, 512
G = GRID_SIZE
P = 128          # SBUF partitions
NH = H // P      # 4 row blocks (h-chunks)
S = 256          # pixels per partition per sub-chunk (w-half)
NW = W // S      # 2 w-halves
N_CORES = 8

_NC = None            # cached Bass module
_LAST_RESULTS = None  # BassKernelResults of the most recent run (for test.py)

# tile-pool buffer counts (module-level so perf sweeps can override)
BUFS = {"xio": 2, "ep": 2, "tmpbp": 2, "wybp": 2, "small": 2}


def _soft_bin_table(n: int, sigma: float) -> np.ndarray:
    """Host-side mirror of reference._soft_bin for the spatial coords."""
    c = np.linspace(0.0, 1.0, n, dtype=np.float32)
    g = np.linspace(0.0, 1.0, G, dtype=np.float32)
    d = c[:, None] - g[None, :]
    w = np.exp(-(d * d) / np.float32(2.0 * sigma * sigma))
    return (w / w.sum(axis=1, keepdims=True)).astype(np.float32)


def _build_nc(repeat: int = 1):
    import concourse.bacc as bacc
    import concourse.mybir as mybir
    from concourse.tile import TileContext

    f32 = mybir.dt.float32
    bf16 = mybir.dt.bfloat16
    AF = mybir.ActivationFunctionType
    AX = mybir.AxisListType

    nc = bacc.Bacc("TRN2", target_bir_lowering=False, debug=False)
    xb = nc.declare_dram_parameter("xb", [C, H, W], f32, isOutput=False)
    wtab = nc.declare_dram_parameter("wtab", [H, G], f32, isOutput=False)
    gv = nc.declare_dram_parameter("gv", [1, G], f32, isOutput=True)
    owx = nc.declare_dram_parameter("owx", [H, W, G], f32, isOutput=True)
    owy = nc.declare_dram_parameter("owy", [H, W, G], f32, isOutput=True)
    owz = nc.declare_dram_parameter("owz", [H, W, G], f32, isOutput=True)

    gvals = np.linspace(0.0, 1.0, G, dtype=np.float32)
    # exp(-(v-g)^2/(2s^2)) = (sqrt(pi)/2) * erf'(k*(v-g)), k = 1/(s*sqrt(2)).
    # The sqrt(pi)/2 factor cancels in the normalization, so one
    # Derivative_Erf activation per bin computes the (unscaled) weights.
    erf_scale = float(np.float32(1.0 / (SIGMA_COLOR * np.sqrt(2.0))))

    with TileContext(nc) as tc:
        with (
            tc.tile_pool(name="const", bufs=1) as constp,
            tc.tile_pool(name="xio", bufs=BUFS["xio"]) as xio,
            tc.tile_pool(name="tmpbp", bufs=BUFS["tmpbp"]) as tmpbp,
            tc.tile_pool(name="ep", bufs=BUFS["ep"]) as ep,
            tc.tile_pool(name="wybp", bufs=BUFS["wybp"]) as wybp,
            tc.tile_pool(name="small", bufs=BUFS["small"]) as smallp,
            tc.tile_pool(name="psum", bufs=1, space="PSUM") as psump,
            tc.tile_pool(name="dram", bufs=1, space="DRAM") as dramp,
        ):
          for _rep in range(repeat):
            # ---- constants, loaded once ----
            wtab4 = constp.tile([P, NH, G], f32)
            nc.sync.dma_start(
                out=wtab4[:], in_=wtab[:].rearrange("(c p) g -> p c g", p=P)
            )
            ones = constp.tile([P, 1], f32)
            nc.vector.memset(ones[:], 1.0)
            onesb = constp.tile([P, 1], bf16)
            nc.vector.memset(onesb[:], 1.0)

            # wx broadcast tile built on-chip: load the flat 32KB table onto
            # one partition, then PE-broadcast to all 128 partitions via a
            # ones-stationary K=1 matmul (single-term sums -> bit-exact copy).
            wrow_sb = constp.tile([P, W * G], f32)
            wrow1 = constp.tile([1, W * G], f32)
            nc.sync.dma_start(out=wrow1[:], in_=wtab[:].rearrange("h g -> (h g)"))
            ones1r = constp.tile([1, P], f32)
            nc.vector.memset(ones1r[:], 1.0)
            NB = 512  # one PSUM bank per broadcast block
            bcast_ps = [
                psump.tile([P, NB], f32, name=f"bc{_rep}_{i}") for i in range(2)
            ]
            for k in range((W * G) // NB):
                bp = bcast_ps[k % 2]
                nc.tensor.matmul(
                    bp[:], lhsT=ones1r[:],
                    rhs=wrow1[:, k * NB : (k + 1) * NB],
                    start=True, stop=True,
                )
                nc.vector.tensor_copy(
                    out=wrow_sb[:, k * NB : (k + 1) * NB], in_=bp[:]
                )
            dbias = constp.tile([P, G], f32)
            for gi in range(G):
                nc.vector.memset(
                    dbias[:, gi : gi + 1],
                    -float(np.float32(erf_scale) * np.float32(gvals[gi])),
                )

            # prefetch whole channels: tile[p, c*W+w] = x[ch, c*P+p, w]
            xch = []
            for ch in range(C):
                xc_t = xio.tile([P, NH * W], f32, name=f"xch{ch}", tag=f"xch{ch}")
                nc.sync.dma_start(
                    out=xc_t[:].rearrange("p (c w) -> p c w", w=W),
                    in_=xb[ch, :, :].rearrange("(c p) w -> p c w", p=P),
                )
                xch.append(xc_t)

            # ---- PSUM accumulators ----
            s_ps = psump.tile([1, W], f32)      # column sums of x0
            typ = psump.tile([1, G], f32)       # term_y
            txp = psump.tile([1, G], f32)       # term_x
            tzp = psump.tile([1, W], f32)       # term_z partials (32 fblk x 16 g)

            tz_started = [False]

            for c in range(NH):
                rows = slice(c * P, (c + 1) * P)

                x0c = xch[0][:, c * W : (c + 1) * W]
                x1c = xch[1][:, c * W : (c + 1) * W]
                x2c = xch[2][:, c * W : (c + 1) * W]

                # term_x partial: S[w] += sum_h x0[h, w]  (ones-stationary)
                nc.tensor.matmul(
                    s_ps[:], lhsT=ones[:], rhs=x0c,
                    start=(c == 0), stop=(c == NH - 1),
                )
                # term_y partial: row sums of x1, dotted with the h table
                s2c = smallp.tile([P, 1], f32, tag="s2")
                nc.vector.reduce_sum(out=s2c[:], in_=x1c, axis=AX.X)
                nc.tensor.matmul(
                    typ[:], lhsT=s2c[:], rhs=wtab4[:, c, :],
                    start=(c == 0), stop=(c == NH - 1),
                )

                if c == NH - 1:
                    # term_x tail: S (1,512) -> DRAM -> (128,4) partition-major.
                    # Emitted here (right after the last colsum matmul) so the
                    # roundtrip overlaps the final chunk's main compute.
                    s_sb = smallp.tile([1, W], f32, tag="ssb")
                    nc.scalar.copy(out=s_sb[:], in_=s_ps[:])
                    stag = dramp.tile([1, W], f32)
                    nc.sync.dma_start(out=stag[:], in_=s_sb[:])
                    sT = smallp.tile([P, NH], f32, tag="sT")
                    nc.sync.dma_start(
                        out=sT[:], in_=stag[0, :].rearrange("(cc p) -> p cc", p=P)
                    )
                    for jc in range(NH):
                        nc.tensor.matmul(
                            txp[:], lhsT=sT[:, jc : jc + 1], rhs=wtab4[:, jc, :],
                            start=(jc == 0), stop=(jc == NH - 1),
                        )

                # wy_full row block: per-partition h-row table broadcast along w
                wyb = wybp.tile([P, S * G], f32, tag="wyb")
                nc.gpsimd.tensor_copy(
                    out=wyb[:].rearrange("p (s g) -> p s g", g=G),
                    in_=wtab4[:, c : c + 1, :].broadcast_to((P, S, G)),
                )
                for j in range(NW):
                    nc.sync.dma_start(
                        out=owy[rows, j * S : (j + 1) * S, :].rearrange(
                            "p w g -> p (w g)"
                        ),
                        in_=wyb[:],
                    )
                # wx_full row block straight from the resident broadcast tile
                nc.sync.dma_start(
                    out=owx[rows, :, :].rearrange("p w g -> p (w g)"),
                    in_=wrow_sb[:],
                )

                segs = [S] * NW
                col0 = 0
                for j, Sj in enumerate(segs):
                    cols = slice(col0, col0 + Sj)
                    col0 += Sj
                    x2h = x2c[:, cols]
                    e = ep.tile([P, S * G], f32, tag="e")
                    e3 = e[:, : Sj * G].rearrange("p (s g) -> p s g", g=G)
                    for gi in range(G):
                        nc.scalar.activation(
                            e3[:, :, gi], x2h, AF.Derivative_Erf,
                            bias=dbias[:, gi : gi + 1], scale=erf_scale,
                        )
                    s_t = smallp.tile([P, S], f32, tag="s")
                    nc.vector.reduce_sum(out=s_t[:, :Sj], in_=e3, axis=AX.X)
                    rcp = smallp.tile([P, S], f32, tag="rcp")
                    nc.vector.reciprocal(rcp[:, :Sj], s_t[:, :Sj])
                    # normalize in place and store as early as possible
                    nc.vector.tensor_mul(
                        e3, e3, rcp[:, :Sj, None].broadcast_to((P, Sj, G))
                    )
                    nc.sync.dma_start(out=owz[rows, cols, :], in_=e3)

                    # term_z: tmpb = wz * x2 (bf16), then PE-reduce over partitions
                    tmpb = tmpbp.tile([P, S * G], bf16, tag="tmpb")
                    nc.vector.tensor_mul(
                        tmpb[:, : Sj * G].rearrange("p (s g) -> p s g", g=G),
                        e3,
                        x2h[:, :, None].broadcast_to((P, Sj, G)),
                    )
                    nblk = (Sj * G) // W  # 512-wide matmul blocks
                    for k in range(nblk):
                        last = (
                            (c == NH - 1)
                            and (j == len(segs) - 1)
                            and (k == nblk - 1)
                        )
                        nc.tensor.matmul(
                            tzp[:], lhsT=onesb[:],
                            rhs=tmpb[:, k * W : (k + 1) * W],
                            start=(not tz_started[0]), stop=last,
                        )
                        tz_started[0] = True


            # ---- final grid_value assembly ----
            # term_z: fold the 32 f-blocks of tzp into 16 g values
            tzs = smallp.tile([1, W], f32, tag="tzs")
            nc.scalar.copy(out=tzs[:], in_=tzp[:])
            tzv = smallp.tile([1, G], f32, tag="tzv")
            tzs3 = tzs[:].rearrange("a (r g) -> a g r", g=G)
            for gi in range(G):
                nc.vector.reduce_sum(
                    out=tzv[:, gi : gi + 1], in_=tzs3[:, gi, :], axis=AX.X
                )

            gv_sb = smallp.tile([1, G], f32, tag="gv")
            nc.scalar.copy(out=gv_sb[:], in_=txp[:])
            nc.vector.tensor_add(gv_sb[:], gv_sb[:], typ[:])
            nc.vector.tensor_add(gv_sb[:], gv_sb[:], tzv[:])
            nc.sync.dma_start(out=gv[:], in_=gv_sb[:])

    nc.compile()
    return nc


def _get_nc():
    global _NC
    if _NC is None:
        _NC = _build_nc()
    return _NC


def kernel(x: np.ndarray):
    global _LAST_RESULTS
    from concourse.bass_utils import run_bass_kernel_spmd

    x = np.ascontiguousarray(np.asarray(x), dtype=np.float32)
    assert x.shape == (B, C, H, W), x.shape

    nc = _get_nc()
    wtab = _soft_bin_table(H, SIGMA_SPATIAL)                      # (512, 16)
    in_maps = [{"xb": x[b], "wtab": wtab} for b in range(B)]

    res = run_bass_kernel_spmd(nc, in_maps, list(range(N_CORES)))
    _LAST_RESULTS = res
    outs = res.results

    grid_value = np.stack([o["gv"] for o in outs]).astype(np.float32)  # (8,1,16)
    wx_full = np.stack([o["owx"] for o in outs])[:, None]
    wy_full = np.stack([o["owy"] for o in outs])[:, None]
    wz_full = np.stack([o["owz"] for o in outs])[:, None]
    return grid_value, wx_full, wy_full, wz_full


# revision 54
# speedup vs baseline: 1.1017x; 1.0051x over previous
"""BilateralGridLayer Trainium2 kernel.

Data-parallel over batch: sample b -> NeuronCore b (8 cores).
Per core, for its (3, 512, 512) sample:
  - wz_full: per-pixel Gaussian soft-binning of channel 2 over a 16-bin grid.
    One Derivative_Erf activation per bin (erf'(x) = 2/sqrt(pi)*exp(-x^2);
    the constant cancels in the normalization), then DVE reduce+recip+mul.
  - wx_full / wy_full: broadcasts of the tiny spatial soft-bin table,
    materialized from SBUF-resident broadcast tiles at DMA line rate.
  - grid_value: weighted reductions using ones-stationary matmuls on the
    TensorEngine; the z-term reduces a bf16 product tile on the PE.
Modeled (instruction-cost timeline) per-core time ~163 us against a
~149 us DMA floor for the 51 MB/core of HBM traffic this layer moves
(48 MB of that is the mandatory output writes; the replicated wx table
is broadcast on-chip via K=1 ones-matmuls instead of a 4 MB DMA load).
"""

import numpy as np

GRID_SIZE = 16
SIGMA_SPATIAL = 0.1
SIGMA_COLOR = 0.1
B, C, H, W = 8, 3, 512, 512
G = GRID_SIZE
P = 128          # SBUF partitions
NH = H // P      # 4 row blocks (h-chunks)
S = 256          # pixels per partition per sub-chunk (w-half)
NW = W // S      # 2 w-halves
N_CORES = 8

_NC = None            # cached Bass module
_LAST_RESULTS = None  # BassKernelResults of the most recent run (for test.py)

# tile-pool buffer counts (module-level so perf sweeps can override)
BUFS = {"xio": 2, "ep": 2, "tmpbp": 2, "wybp": 2, "small": 2}


def _soft_bin_table(n: int, sigma: float) -> np.ndarray:
    """Host-side mirror of reference._soft_bin for the spatial coords."""
    c = np.linspace(0.0, 1.0, n, dtype=np.float32)
    g = np.linspace(0.0, 1.0, G, dtype=np.float32)
    d = c[:, None] - g[None, :]
    w = np.exp(-(d * d) / np.float32(2.0 * sigma * sigma))
    return (w / w.sum(axis=1, keepdims=True)).astype(np.float32)


def _build_nc(repeat: int = 1):
    import concourse.bacc as bacc
    import concourse.mybir as mybir
    from concourse.tile import TileContext

    f32 = mybir.dt.float32
    bf16 = mybir.dt.bfloat16
    AF = mybir.ActivationFunctionType
    AX = mybir.AxisListType

    nc = bacc.Bacc("TRN2", target_bir_lowering=False, debug=False)
    xb = nc.declare_dram_parameter("xb", [C, H, W], f32, isOutput=False)
    wtab = nc.declare_dram_parameter("wtab", [H, G], f32, isOutput=False)
    gv = nc.declare_dram_parameter("gv", [1, G], f32, isOutput=True)
    owx = nc.declare_dram_parameter("owx", [H, W, G], f32, isOutput=True)
    owy = nc.declare_dram_parameter("owy", [H, W, G], f32, isOutput=True)
    owz = nc.declare_dram_parameter("owz", [H, W, G], f32, isOutput=True)

    gvals = np.linspace(0.0, 1.0, G, dtype=np.float32)
    # exp(-(v-g)^2/(2s^2)) = (sqrt(pi)/2) * erf'(k*(v-g)), k = 1/(s*sqrt(2)).
    # The sqrt(pi)/2 factor cancels in the normalization, so one
    # Derivative_Erf activation per bin computes the (unscaled) weights.
    erf_scale = float(np.float32(1.0 / (SIGMA_COLOR * np.sqrt(2.0))))

    with TileContext(nc) as tc:
        with (
            tc.tile_pool(name="const", bufs=1) as constp,
            tc.tile_pool(name="xio", bufs=BUFS["xio"]) as xio,
            tc.tile_pool(name="tmpbp", bufs=BUFS["tmpbp"]) as tmpbp,
            tc.tile_pool(name="ep", bufs=BUFS["ep"]) as ep,
            tc.tile_pool(name="wybp", bufs=BUFS["wybp"]) as wybp,
            tc.tile_pool(name="small", bufs=BUFS["small"]) as smallp,
            tc.tile_pool(name="psum", bufs=1, space="PSUM") as psump,
            tc.tile_pool(name="dram", bufs=1, space="DRAM") as dramp,
        ):
          for _rep in range(repeat):
            # ---- constants, loaded once ----
            wtab4 = constp.tile([P, NH, G], f32)
            nc.sync.dma_start(
                out=wtab4[:], in_=wtab[:].rearrange("(c p) g -> p c g", p=P)
            )
            ones = constp.tile([P, 1], f32)
            nc.vector.memset(ones[:], 1.0)
            onesb = constp.tile([P, 1], bf16)
            nc.vector.memset(onesb[:], 1.0)

            # wx broadcast tile built on-chip: load the flat 32KB table onto
            # one partition, then PE-broadcast to all 128 partitions via a
            # ones-stationary K=1 matmul (single-term sums -> bit-exact copy).
            wrow_sb = constp.tile([P, W * G], f32)
            wrow1 = constp.tile([1, W * G], f32)
            nc.sync.dma_start(out=wrow1[:], in_=wtab[:].rearrange("h g -> (h g)"))
            ones1r = constp.tile([1, P], f32)
            nc.vector.memset(ones1r[:], 1.0)
            NB = 512  # one PSUM bank per broadcast block
            bcast_ps = [
                psump.tile([P, NB], f32, name=f"bc{_rep}_{i}") for i in range(2)
            ]
            for k in range((W * G) // NB):
                bp = bcast_ps[k % 2]
                nc.tensor.matmul(
                    bp[:], lhsT=ones1r[:],
                    rhs=wrow1[:, k * NB : (k + 1) * NB],
                    start=True, stop=True,
                )
                nc.vector.tensor_copy(
                    out=wrow_sb[:, k * NB : (k + 1) * NB], in_=bp[:]
                )
            dbias = constp.tile([P, G], f32)
            for gi in range(G):
                nc.vector.memset(
                    dbias[:, gi : gi + 1],
                    -float(np.float32(erf_scale) * np.float32(gvals[gi])),
                )

            # prefetch whole channels: tile[p, c*W+w] = x[ch, c*P+p, w].
            # x2 (the wz input) first, with its first row block as its own
            # DMA so the first activations start after 256KB, not 3MB.
            xch = [None, None, None]
            for ch in (2, 0, 1):
                xc_t = xio.tile([P, NH * W], f32, name=f"xch{ch}", tag=f"xch{ch}")
                xc3 = xc_t[:].rearrange("p (c w) -> p c w", w=W)
                xb3 = xb[ch, :, :].rearrange("(c p) w -> p c w", p=P)
                if ch == 2:
                    nc.sync.dma_start(out=xc3[:, 0, :], in_=xb3[:, 0, :])
                    nc.sync.dma_start(out=xc3[:, 1:, :], in_=xb3[:, 1:, :])
                else:
                    nc.sync.dma_start(out=xc3, in_=xb3)
                xch[ch] = xc_t

            # ---- PSUM accumulators ----
            s_ps = psump.tile([1, W], f32)      # column sums of x0
            typ = psump.tile([1, G], f32)       # term_y
            txp = psump.tile([1, G], f32)       # term_x
            tzp = psump.tile([1, W], f32)       # term_z partials (32 fblk x 16 g)

            tz_started = [False]

            for c in range(NH):
                rows = slice(c * P, (c + 1) * P)

                x0c = xch[0][:, c * W : (c + 1) * W]
                x1c = xch[1][:, c * W : (c + 1) * W]
                x2c = xch[2][:, c * W : (c + 1) * W]

                # term_x partial: S[w] += sum_h x0[h, w]  (ones-stationary)
                nc.tensor.matmul(
                    s_ps[:], lhsT=ones[:], rhs=x0c,
                    start=(c == 0), stop=(c == NH - 1),
                )
                # term_y partial: row sums of x1, dotted with the h table
                s2c = smallp.tile([P, 1], f32, tag="s2")
                nc.vector.reduce_sum(out=s2c[:], in_=x1c, axis=AX.X)
                nc.tensor.matmul(
                    typ[:], lhsT=s2c[:], rhs=wtab4[:, c, :],
                    start=(c == 0), stop=(c == NH - 1),
                )

                if c == NH - 1:
                    # term_x tail: S (1,512) -> DRAM -> (128,4) partition-major.
                    # Emitted here (right after the last colsum matmul) so the
                    # roundtrip overlaps the final chunk's main compute.
                    s_sb = smallp.tile([1, W], f32, tag="ssb")
                    nc.scalar.copy(out=s_sb[:], in_=s_ps[:])
                    stag = dramp.tile([1, W], f32)
                    nc.sync.dma_start(out=stag[:], in_=s_sb[:])
                    sT = smallp.tile([P, NH], f32, tag="sT")
                    nc.sync.dma_start(
                        out=sT[:], in_=stag[0, :].rearrange("(cc p) -> p cc", p=P)
                    )
                    for jc in range(NH):
                        nc.tensor.matmul(
                            txp[:], lhsT=sT[:, jc : jc + 1], rhs=wtab4[:, jc, :],
                            start=(jc == 0), stop=(jc == NH - 1),
                        )

                # wy_full row block: per-partition h-row table broadcast along w
                wyb = wybp.tile([P, S * G], f32, tag="wyb")
                nc.gpsimd.tensor_copy(
                    out=wyb[:].rearrange("p (s g) -> p s g", g=G),
                    in_=wtab4[:, c : c + 1, :].broadcast_to((P, S, G)),
                )
                for j in range(NW):
                    nc.sync.dma_start(
                        out=owy[rows, j * S : (j + 1) * S, :].rearrange(
                            "p w g -> p (w g)"
                        ),
                        in_=wyb[:],
                    )
                # wx_full row block straight from the resident broadcast tile
                nc.sync.dma_start(
                    out=owx[rows, :, :].rearrange("p w g -> p (w g)"),
                    in_=wrow_sb[:],
                )

                segs = [S] * NW
                col0 = 0
                for j, Sj in enumerate(segs):
                    cols = slice(col0, col0 + Sj)
                    col0 += Sj
                    x2h = x2c[:, cols]
                    e = ep.tile([P, S * G], f32, tag="e")
                    e3 = e[:, : Sj * G].rearrange("p (s g) -> p s g", g=G)
                    for gi in range(G):
                        nc.scalar.activation(
                            e3[:, :, gi], x2h, AF.Derivative_Erf,
                            bias=dbias[:, gi : gi + 1], scale=erf_scale,
                        )
                    s_t = smallp.tile([P, S], f32, tag="s")
                    nc.vector.reduce_sum(out=s_t[:, :Sj], in_=e3, axis=AX.X)
                    rcp = smallp.tile([P, S], f32, tag="rcp")
                    nc.vector.reciprocal(rcp[:, :Sj], s_t[:, :Sj])
                    # normalize in place and store as early as possible
                    nc.vector.tensor_mul(
                        e3, e3, rcp[:, :Sj, None].broadcast_to((P, Sj, G))
                    )
                    nc.sync.dma_start(out=owz[rows, cols, :], in_=e3)

                    # term_z: tmpb = wz * x2 (bf16), then PE-reduce over partitions
                    tmpb = tmpbp.tile([P, S * G], bf16, tag="tmpb")
                    nc.vector.tensor_mul(
                        tmpb[:, : Sj * G].rearrange("p (s g) -> p s g", g=G),
                        e3,
                        x2h[:, :, None].broadcast_to((P, Sj, G)),
                    )
                    nblk = (Sj * G) // W  # 512-wide matmul blocks
                    for k in range(nblk):
                        last = (
                            (c == NH - 1)
                            and (j == len(segs) - 1)
                            and (k == nblk - 1)
                        )
                        nc.tensor.matmul(
                            tzp[:], lhsT=onesb[:],
                            rhs=tmpb[:, k * W : (k + 1) * W],
                            start=(not tz_started[0]), stop=last,
                        )
                        tz_started[0] = True


            # ---- final grid_value assembly ----
            # term_z: fold the 32 f-blocks of tzp into 16 g values
            tzs = smallp.tile([1, W], f32, tag="tzs")
            nc.scalar.copy(out=tzs[:], in_=tzp[:])
            tzv = smallp.tile([1, G], f32, tag="tzv")
            tzs3 = tzs[:].rearrange("a (r g) -> a g r", g=G)
            for gi in range(G):
                nc.vector.reduce_sum(
                    out=tzv[:, gi : gi + 1], in_=tzs3[:, gi, :], axis=AX.X
                )

            gv_sb = smallp.tile([1, G], f32, tag="gv")
            nc.scalar.copy(out=gv_sb[:], in_=txp[:])
            nc.vector.tensor_add(gv_sb[:], gv_sb[:], typ[:])
            nc.vector.tensor_add(gv_sb[:], gv_sb[:], tzv[:])
            nc.sync.dma_start(out=gv[:], in_=gv_sb[:])

    nc.compile()
    return nc


def _get_nc():
    global _NC
    if _NC is None:
        _NC = _build_nc()
    return _NC


def kernel(x: np.ndarray):
    global _LAST_RESULTS
    from concourse.bass_utils import run_bass_kernel_spmd

    x = np.ascontiguousarray(np.asarray(x), dtype=np.float32)
    assert x.shape == (B, C, H, W), x.shape

    nc = _get_nc()
    wtab = _soft_bin_table(H, SIGMA_SPATIAL)                      # (512, 16)
    in_maps = [{"xb": x[b], "wtab": wtab} for b in range(B)]

    res = run_bass_kernel_spmd(nc, in_maps, list(range(N_CORES)))
    _LAST_RESULTS = res
    outs = res.results

    grid_value = np.stack([o["gv"] for o in outs]).astype(np.float32)  # (8,1,16)
    wx_full = np.stack([o["owx"] for o in outs])[:, None]
    wy_full = np.stack([o["owy"] for o in outs])[:, None]
    wz_full = np.stack([o["owz"] for o in outs])[:, None]
    return grid_value, wx_full, wy_full, wz_full
